# revision 15
# baseline (speedup 1.0000x reference)
"""Quantized 3x3 conv (8-bit symmetric STE quantization of x and w, then
stride-1 pad-1 conv) on 8 Trainium2 NeuronCores.

Strategy (v7)
-------------
Data-parallel over batch: 4 images per core (32/8).  The quantization is
integer-exact, so it is hoisted to the host:
  * x is quantized host-side to integers kx in [-127,127] (reproducing
    jnp.round(x/step) bit-exactly: fp32 divide + round-half-even + clip),
    packed into zero-padded 58x58 bf16 grids, and DMA'd directly into the
    matmul operand layout — no on-device quantization or upcast at all.
  * w is quantized host-side (tiny) to integers kw, laid out as
    lhsT [ci, tap, co] bf16 and duplicated into both partition halves.
Per core:
  * conv = 9 shifted matmuls (K=ci=64, M=co=128) accumulating in PSUM.
    Integer products accumulate exactly in fp32 PSUM (|sum| <= 9.3e6 < 2^24).
    Two images run concurrently on the PE via row-group tiling: image (2g)
    on partitions 0-63, image (2g+1) on partitions 64-127 (the PE streams
    both row groups' columns concurrently -> full 128-row utilization).
    The rhs uses a 2D [8 rows x 56 cols] access pattern so only the 448
    valid columns stream through the PE (not the 464 incl. row pads).
  * input DMAs are chunked by grid rows; the first, latency-critical chunk
    avoids the late-starting queue-hosting DMA engine via a 120/8 partition
    split, and PE warmup matmuls bridge the head so the HAM clock ramp is
    not reset by an idle gap.
  * PSUM -> SBUF copy applies the final scale s2 = step_x*step_w and stores
    bf16 (bf16 rounding is ~2^-9 relative; the harness gate is 2e-2);
    outputs DMA back at block-group boundaries, even images on the SP
    queue, odd images on the ACT queue, and are upcast to fp32 on the host.
"""

import numpy as np
import ml_dtypes

import concourse.bass as bass
import concourse.mybir as mybir
import concourse.tile as tile
from concourse import bacc
from concourse.bass_utils import run_bass_kernel_spmd

dt = mybir.dt

N_CORES = 8
NPC = 4                # images per core
CI, CO = 64, 128
H = W = 56
WP = 58                # padded row width (56 + 2)
LEAD = 4               # guard elems before the padded grid
IMG_ELEMS = LEAD + WP * WP + 8   # 4 + 3364 + 8 = 3376
PACK = H * W           # 3136
H0S = [1 + 8 * i for i in range(7)]   # padded-row start of each 8-row block
N_WARM = 26            # PE warmup matmuls (HAM clock-ramp bridge)

# grid-row boundaries of the input DMA chunks
RB0 = [0, 9, 12, 26, 42, 58]   # pair 0: small first chunks (block 0 unblocks)
RB1 = [0, 18, 42, 58]          # pair 1: latency-insensitive
ITERS = [[0], [1, 2], [3, 4], [5], [6]]
# output DMA boundaries: after ITERS group gi, blocks [b0, b1) are complete
OUT_CHUNKS = {1: (0, 3), 2: (3, 5), 3: (5, 6), 4: (6, 7)}

_PROG_CACHE = {}


def _chunk_cols(rb, ci):
    c0 = 0 if ci == 0 else LEAD + WP * rb[ci]
    c1 = IMG_ELEMS if ci == len(rb) - 2 else LEAD + WP * rb[ci + 1]
    return c0, c1


def _build_program(s2):
    """One SPMD program; per-core shards differ only through in_maps.

    s2 (=step_x*step_w) is embedded as an immediate — the program is
    specialized per (alpha_x, alpha_w) value and cached."""
    s2 = float(np.float32(s2))
    nc = bacc.Bacc(None)
    x_in = nc.declare_dram_parameter("xg", [2 * 128, IMG_ELEMS], dt.bfloat16,
                                     isOutput=False)
    wq_in = nc.declare_dram_parameter("wq", [128, 9, CO], dt.bfloat16,
                                      isOutput=False)
    out = nc.declare_dram_parameter("out", [NPC * CO, PACK], dt.bfloat16,
                                    isOutput=True)

    with tile.TileContext(nc) as tc:
        with (
            tc.tile_pool(name="sb", bufs=1) as sb,
            tc.tile_pool(name="ps", bufs=4, space="PSUM") as psp,
        ):
            wq = sb.tile([128, 9, CO], dt.bfloat16)
            dummy = sb.tile([128, 128], dt.bfloat16)
            xq = [sb.tile([128, IMG_ELEMS], dt.bfloat16, name=f"xq{g}", tag=f"xq{g}")
                  for g in range(2)]
            os_ = [sb.tile([128, PACK], dt.bfloat16, name=f"os{n}", tag=f"os{n}")
                   for n in range(NPC)]

            # warmup fodder must be initialized before the PE touches it
            nc.gpsimd.memset(dummy[:], 0.0)

            def x_dma(eng, g, rb, ci, split=False):
                c0, c1 = _chunk_cols(rb, ci)
                if split:
                    # the 16th DMA-engine slice (partitions 120-127) would
                    # land on the queue-hosting engine, which comes up ~2 us
                    # late at kernel start — split so the first chunk avoids
                    # it (small slice first so it isn't FIFO'd behind).
                    eng.dma_start(out=xq[g][120:128, c0:c1],
                                  in_=x_in[128 * g + 120:128 * (g + 1), c0:c1])
                    eng.dma_start(out=xq[g][0:120, c0:c1],
                                  in_=x_in[128 * g:128 * g + 120, c0:c1])
                else:
                    eng.dma_start(out=xq[g][:, c0:c1],
                                  in_=x_in[128 * g:128 * (g + 1), c0:c1])

            # weights: tap 0 + taps 1-4 early on the ACT queue (concurrent
            # with pair-0 chunk 0 on the SP queue); taps 5-8 mid-chain on SP
            # so their bytes don't contend with the first chunk.
            nc.scalar.dma_start(out=wq[:, 0:1, :], in_=wq_in[:, 0:1, :])
            nc.scalar.dma_start(out=wq[:, 1:5, :], in_=wq_in[:, 1:5, :])
            x_dma(nc.sync, 0, RB0, 0, split=True)
            x_dma(nc.sync, 0, RB0, 1)
            x_dma(nc.sync, 0, RB0, 2)
            nc.sync.dma_start(out=wq[:, 5:9, :], in_=wq_in[:, 5:9, :])
            for ci in range(3, len(RB0) - 1):
                x_dma(nc.sync, 0, RB0, ci)
            for ci in range(len(RB1) - 1):
                x_dma(nc.sync, 1, RB1, ci)

            # PE warmup (HAM clock-ramp) overlapping the DMA head.
            warm = psp.tile([128, 512], dt.float32, name="warm", tag="ps")
            for _ in range(N_WARM):
                nc.tensor.matmul(
                    warm[:, 0:128], lhsT=dummy[0:64, 0:128],
                    rhs=dummy[0:64, 0:128], start=True, stop=True,
                )
            # DCE guard; target is overwritten by the real img-0 scale pass
            nc.vector.tensor_copy(os_[0][0:1, 0:1], warm[0:1, 0:1])

            for g in range(2):
                # 7 blocks of 8 output rows, processed in ITERS groups so
                # one PSUM tile spans <=2 banks; images 2g / 2g+1 concurrently
                # via PE row-group tiling (partition halves).
                for gi, blocks in enumerate(ITERS):
                    b0, nb = blocks[0], len(blocks)
                    ps_pair = [psp.tile([128, 1024], dt.float32,
                                        name=f"psum_g{g}b{b0}h{h}", tag="ps")
                               for h in range(2)]
                    # each block sits bank-aligned (cols 0 and 512)
                    ps2 = [p.rearrange("p (b x) -> p b x", b=2) for p in ps_pair]
                    for t in range(9):
                        dh, dw = t // 3, t % 3
                        for h in (1, 0):
                            for bi in range(nb):
                                off = LEAD + (H0S[b0 + bi] + dh - 1) * WP + dw
                                rhs = xq[g][64 * h:64 * (h + 1),
                                            off:off + 8 * WP].rearrange(
                                    "p (r c) -> p r c", c=WP)[:, :, 0:56]
                                nc.tensor.matmul(
                                    ps2[h][:, bi, 0:448],
                                    lhsT=wq[64 * h:64 * (h + 1), t, :],
                                    rhs=rhs,
                                    start=(t == 0), stop=(t == 8),
                                )
                    # scale -> bf16 (flat, pads already excluded); even image
                    # on DVE, odd image on ACT (both otherwise idle here).
                    for h in range(2):
                        img = 2 * g + h
                        sel = ps2[h][:, 0:nb, 0:448]
                        dst = os_[img][:, 448 * b0:448 * (b0 + nb)]
                        if h == 0:
                            nc.vector.tensor_scalar_mul(
                                out=dst, in0=sel, scalar1=s2)
                        else:
                            nc.scalar.activation(
                                out=dst, in_=sel,
                                func=mybir.ActivationFunctionType.Copy,
                                scale=s2,
                            )
                    # output DMA at block-group boundaries; even images on
                    # the SP queue, odd images on the ACT queue (parallel
                    # descriptor issue).
                    if gi in OUT_CHUNKS:
                        ob0, ob1 = OUT_CHUNKS[gi]
                        for h in range(2):
                            img = 2 * g + h
                            eng = nc.sync if h == 0 else nc.scalar
                            eng.dma_start(
                                out=out[CO * img:CO * (img + 1),
                                        448 * ob0:448 * ob1],
                                in_=os_[img][:, 448 * ob0:448 * ob1],
                            )
    if not nc.is_finalized():
        nc.finalize()   # Bacc: runs wait-splitting + register allocation
    return nc


def _host_prep(x, w, alpha_x, alpha_w):
    """Scalar/weight/activation prep, replicating the reference's fp32
    arithmetic exactly (fp32 divide + round-half-even + clip)."""
    x = np.asarray(x, dtype=np.float32)
    w = np.asarray(w, dtype=np.float32)
    ax = np.float32(max(np.float32(np.asarray(alpha_x).reshape(-1)[0]), np.float32(0)))
    aw = np.float32(max(np.float32(np.asarray(alpha_w).reshape(-1)[0]), np.float32(0)))
    step_x = np.float32(np.float32(np.float32(2.0) * ax) / np.float32(254.0))
    step_w = np.float32(np.float32(np.float32(2.0) * aw) / np.float32(254.0))
    s2 = np.float32(step_x * step_w)

    with np.errstate(divide="ignore", invalid="ignore"):
        kx = np.clip(np.round(x / step_x), -127, 127)
        kw = np.clip(np.round((w / step_w).astype(np.float32)), -127, 127)
    kx = np.nan_to_num(kx, nan=0.0, posinf=127.0, neginf=-127.0)

    # pack into zero-padded 58x58 bf16 grids with LEAD/tail guards
    n = x.shape[0]
    grids = np.zeros((n, CI, IMG_ELEMS), dtype=ml_dtypes.bfloat16)
    gv = grids[:, :, LEAD:LEAD + WP * WP].reshape(n, CI, WP, WP)
    gv[:, :, 1:57, 1:57] = kx.reshape(n, CI, H, W)

    # weight quantization, integers in fp32 (exactly the reference math)
    kw = np.nan_to_num(kw, nan=0.0, posinf=127.0, neginf=-127.0)
    kw = kw.astype(np.float32).reshape(CO, CI, 9).transpose(1, 2, 0)  # [ci,tap,co]
    wq = np.concatenate([kw, kw], axis=0).astype(ml_dtypes.bfloat16)
    return grids, wq, s2


def _in_maps(grids, wq):
    return [
        {
            "xg": grids[NPC * c:NPC * (c + 1)].reshape(2 * 128, IMG_ELEMS),
            "wq": wq,
        }
        for c in range(N_CORES)
    ]


def get_program(s2=float(np.float32(np.float32(2.0 / 254.0) ** 2))):
    key = float(np.float32(s2))
    if key not in _PROG_CACHE:
        _PROG_CACHE[key] = _build_program(key)
    return _PROG_CACHE[key]


def run_on_hw(x, w, alpha_x, alpha_w, trace=False):
    grids, wq, s2 = _host_prep(x, w, alpha_x, alpha_w)
    nc = get_program(s2)
    res = run_bass_kernel_spmd(nc, _in_maps(grids, wq),
                               list(range(N_CORES)), trace=trace)
    out = np.concatenate(
        [np.asarray(res.results[i]["out"]).astype(np.float32).reshape(NPC, CO, H, W)
         for i in range(N_CORES)], axis=0)
    return out, res


def kernel(x, w, alpha_x, alpha_w):
    out, _ = run_on_hw(x, w, alpha_x, alpha_w)
    return out


# revision 17
# speedup vs baseline: 1.0247x; 1.0247x over previous
"""Quantized 3x3 conv (8-bit symmetric STE quantization of x and w, then
stride-1 pad-1 conv) on 8 Trainium2 NeuronCores.

Strategy (v7)
-------------
Data-parallel over batch: 4 images per core (32/8).  The quantization is
integer-exact, so it is hoisted to the host:
  * x is quantized host-side to integers kx in [-127,127] (reproducing
    jnp.round(x/step) bit-exactly: fp32 divide + round-half-even + clip),
    packed into zero-padded 58x58 bf16 grids, and DMA'd directly into the
    matmul operand layout — no on-device quantization or upcast at all.
  * w is quantized host-side (tiny) to integers kw, laid out as
    lhsT [ci, tap, co] bf16 and duplicated into both partition halves.
Per core:
  * conv = 9 shifted matmuls (K=ci=64, M=co=128) accumulating in PSUM.
    Integer products accumulate exactly in fp32 PSUM (|sum| <= 9.3e6 < 2^24).
    Two images run concurrently on the PE via row-group tiling: image (2g)
    on partitions 0-63, image (2g+1) on partitions 64-127 (the PE streams
    both row groups' columns concurrently -> full 128-row utilization).
    The rhs uses a 2D [8 rows x 56 cols] access pattern so only the 448
    valid columns stream through the PE (not the 464 incl. row pads).
  * input DMAs are chunked by grid rows; the first, latency-critical chunk
    avoids the late-starting queue-hosting DMA engine via a 120/8 partition
    split, and PE warmup matmuls bridge the head so the HAM clock ramp is
    not reset by an idle gap.
  * PSUM -> SBUF copy applies the final scale s2 = step_x*step_w and stores
    bf16 (bf16 rounding is ~2^-9 relative; the harness gate is 2e-2);
    outputs DMA back at block-group boundaries, even images on the SP
    queue, odd images on the ACT queue, and are upcast to fp32 on the host.
"""

import numpy as np
import ml_dtypes

import concourse.bass as bass
import concourse.mybir as mybir
import concourse.tile as tile
from concourse import bacc
from concourse.bass_utils import run_bass_kernel_spmd

dt = mybir.dt

N_CORES = 8
NPC = 4                # images per core
CI, CO = 64, 128
H = W = 56
WP = 58                # padded row width (56 + 2)
LEAD = 4               # guard elems before the padded grid
IMG_ELEMS = LEAD + WP * WP + 8   # 4 + 3364 + 8 = 3376
PACK = H * W           # 3136
H0S = [1 + 8 * i for i in range(7)]   # padded-row start of each 8-row block
N_WARM = 24            # PE warmup matmuls (HAM clock-ramp bridge)

# grid-row boundaries of the input DMA chunks
RB0 = [0, 9, 12, 26, 42, 58]   # pair 0: small first chunks (block 0 unblocks)
RB1 = [0, 18, 42, 58]          # pair 1: latency-insensitive
ITERS = [[0], [1, 2], [3, 4], [5], [6]]
# output DMA boundaries: after ITERS group gi, blocks [b0, b1) are complete
OUT_CHUNKS = {1: (0, 3), 2: (3, 5), 3: (5, 6), 4: (6, 7)}

_PROG_CACHE = {}


def _chunk_cols(rb, ci):
    c0 = 0 if ci == 0 else LEAD + WP * rb[ci]
    c1 = IMG_ELEMS if ci == len(rb) - 2 else LEAD + WP * rb[ci + 1]
    return c0, c1


def _build_program(s2):
    """One SPMD program; per-core shards differ only through in_maps.

    s2 (=step_x*step_w) is embedded as an immediate — the program is
    specialized per (alpha_x, alpha_w) value and cached."""
    s2 = float(np.float32(s2))
    nc = bacc.Bacc(None)
    x_in = nc.declare_dram_parameter("xg", [2 * 128, IMG_ELEMS], dt.bfloat16,
                                     isOutput=False)
    wq_in = nc.declare_dram_parameter("wq", [128, 9, CO], dt.bfloat16,
                                      isOutput=False)
    out = nc.declare_dram_parameter("out", [NPC * CO, PACK], dt.bfloat16,
                                    isOutput=True)

    with tile.TileContext(nc) as tc:
        with (
            tc.tile_pool(name="sb", bufs=1) as sb,
            tc.tile_pool(name="ps", bufs=4, space="PSUM") as psp,
        ):
            wq = sb.tile([128, 9, CO], dt.bfloat16)
            dummy = sb.tile([128, 128], dt.bfloat16)
            xq = [sb.tile([128, IMG_ELEMS], dt.bfloat16, name=f"xq{g}", tag=f"xq{g}")
                  for g in range(2)]
            os_ = [sb.tile([128, PACK], dt.bfloat16, name=f"os{n}", tag=f"os{n}")
                   for n in range(NPC)]

            # warmup fodder must be initialized before the PE touches it
            nc.gpsimd.memset(dummy[:], 0.0)

            def x_dma(eng, g, rb, ci, split=False):
                c0, c1 = _chunk_cols(rb, ci)
                if split:
                    # partition-half split: the h=1 row group's half first
                    # (the first matmul reads it), halving the bytes ahead
                    # of the first matmul.  64-row DMAs also stripe onto
                    # DMA engines 0-7 only, avoiding the queue-hosting
                    # 16th engine which comes up ~2 us late at kernel start.
                    eng.dma_start(out=xq[g][64:128, c0:c1],
                                  in_=x_in[128 * g + 64:128 * (g + 1), c0:c1])
                    eng.dma_start(out=xq[g][0:64, c0:c1],
                                  in_=x_in[128 * g:128 * g + 64, c0:c1])
                else:
                    eng.dma_start(out=xq[g][:, c0:c1],
                                  in_=x_in[128 * g:128 * (g + 1), c0:c1])

            # weights: tap 0 + taps 1-4 early on the ACT queue (concurrent
            # with pair-0 chunk 0 on the SP queue); taps 5-8 mid-chain on SP
            # so their bytes don't contend with the first chunk.
            nc.scalar.dma_start(out=wq[:, 0:1, :], in_=wq_in[:, 0:1, :])
            nc.scalar.dma_start(out=wq[:, 1:5, :], in_=wq_in[:, 1:5, :])
            x_dma(nc.sync, 0, RB0, 0, split=True)
            x_dma(nc.sync, 0, RB0, 1)
            x_dma(nc.sync, 0, RB0, 2)
            nc.sync.dma_start(out=wq[:, 5:9, :], in_=wq_in[:, 5:9, :])
            for ci in range(3, len(RB0) - 1):
                x_dma(nc.sync, 0, RB0, ci)
            for ci in range(len(RB1) - 1):
                x_dma(nc.sync, 1, RB1, ci)

            # PE warmup (HAM clock-ramp) overlapping the DMA head.
            warm = psp.tile([128, 512], dt.float32, name="warm", tag="ps")
            for _ in range(N_WARM):
                nc.tensor.matmul(
                    warm[:, 0:128], lhsT=dummy[0:64, 0:128],
                    rhs=dummy[0:64, 0:128], start=True, stop=True,
                )
            # DCE guard; target is overwritten by the real img-0 scale pass
            nc.vector.tensor_copy(os_[0][0:1, 0:1], warm[0:1, 0:1])

            for g in range(2):
                # 7 blocks of 8 output rows, processed in ITERS groups so
                # one PSUM tile spans <=2 banks; images 2g / 2g+1 concurrently
                # via PE row-group tiling (partition halves).
                for gi, blocks in enumerate(ITERS):
                    b0, nb = blocks[0], len(blocks)
                    ps_pair = [psp.tile([128, 1024], dt.float32,
                                        name=f"psum_g{g}b{b0}h{h}", tag="ps")
                               for h in range(2)]
                    # each block sits bank-aligned (cols 0 and 512)
                    ps2 = [p.rearrange("p (b x) -> p b x", b=2) for p in ps_pair]
                    for t in range(9):
                        dh, dw = t // 3, t % 3
                        for h in (1, 0):
                            for bi in range(nb):
                                off = LEAD + (H0S[b0 + bi] + dh - 1) * WP + dw
                                rhs = xq[g][64 * h:64 * (h + 1),
                                            off:off + 8 * WP].rearrange(
                                    "p (r c) -> p r c", c=WP)[:, :, 0:56]
                                nc.tensor.matmul(
                                    ps2[h][:, bi, 0:448],
                                    lhsT=wq[64 * h:64 * (h + 1), t, :],
                                    rhs=rhs,
                                    start=(t == 0), stop=(t == 8),
                                )
                    # scale -> bf16 (flat, pads already excluded); even image
                    # on DVE, odd image on ACT (both otherwise idle here).
                    for h in range(2):
                        img = 2 * g + h
                        sel = ps2[h][:, 0:nb, 0:448]
                        dst = os_[img][:, 448 * b0:448 * (b0 + nb)]
                        if h == 0:
                            nc.vector.tensor_scalar_mul(
                                out=dst, in0=sel, scalar1=s2)
                        else:
                            nc.scalar.activation(
                                out=dst, in_=sel,
                                func=mybir.ActivationFunctionType.Copy,
                                scale=s2,
                            )
                    # output DMA at block-group boundaries; even images on
                    # the SP queue, odd images on the ACT queue (parallel
                    # descriptor issue).
                    if gi in OUT_CHUNKS:
                        ob0, ob1 = OUT_CHUNKS[gi]
                        for h in range(2):
                            img = 2 * g + h
                            eng = nc.sync if h == 0 else nc.scalar
                            eng.dma_start(
                                out=out[CO * img:CO * (img + 1),
                                        448 * ob0:448 * ob1],
                                in_=os_[img][:, 448 * ob0:448 * ob1],
                            )
    if not nc.is_finalized():
        nc.finalize()   # Bacc: runs wait-splitting + register allocation
    return nc


def _host_prep(x, w, alpha_x, alpha_w):
    """Scalar/weight/activation prep, replicating the reference's fp32
    arithmetic exactly (fp32 divide + round-half-even + clip)."""
    x = np.asarray(x, dtype=np.float32)
    w = np.asarray(w, dtype=np.float32)
    ax = np.float32(max(np.float32(np.asarray(alpha_x).reshape(-1)[0]), np.float32(0)))
    aw = np.float32(max(np.float32(np.asarray(alpha_w).reshape(-1)[0]), np.float32(0)))
    step_x = np.float32(np.float32(np.float32(2.0) * ax) / np.float32(254.0))
    step_w = np.float32(np.float32(np.float32(2.0) * aw) / np.float32(254.0))
    s2 = np.float32(step_x * step_w)

    with np.errstate(divide="ignore", invalid="ignore"):
        kx = np.clip(np.round(x / step_x), -127, 127)
        kw = np.clip(np.round((w / step_w).astype(np.float32)), -127, 127)
    kx = np.nan_to_num(kx, nan=0.0, posinf=127.0, neginf=-127.0)

    # pack into zero-padded 58x58 bf16 grids with LEAD/tail guards
    n = x.shape[0]
    grids = np.zeros((n, CI, IMG_ELEMS), dtype=ml_dtypes.bfloat16)
    gv = grids[:, :, LEAD:LEAD + WP * WP].reshape(n, CI, WP, WP)
    gv[:, :, 1:57, 1:57] = kx.reshape(n, CI, H, W)

    # weight quantization, integers in fp32 (exactly the reference math)
    kw = np.nan_to_num(kw, nan=0.0, posinf=127.0, neginf=-127.0)
    kw = kw.astype(np.float32).reshape(CO, CI, 9).transpose(1, 2, 0)  # [ci,tap,co]
    wq = np.concatenate([kw, kw], axis=0).astype(ml_dtypes.bfloat16)
    return grids, wq, s2


def _in_maps(grids, wq):
    return [
        {
            "xg": grids[NPC * c:NPC * (c + 1)].reshape(2 * 128, IMG_ELEMS),
            "wq": wq,
        }
        for c in range(N_CORES)
    ]


def get_program(s2=float(np.float32(np.float32(2.0 / 254.0) ** 2))):
    key = float(np.float32(s2))
    if key not in _PROG_CACHE:
        _PROG_CACHE[key] = _build_program(key)
    return _PROG_CACHE[key]


def run_on_hw(x, w, alpha_x, alpha_w, trace=False):
    grids, wq, s2 = _host_prep(x, w, alpha_x, alpha_w)
    nc = get_program(s2)
    res = run_bass_kernel_spmd(nc, _in_maps(grids, wq),
                               list(range(N_CORES)), trace=trace)
    out = np.concatenate(
        [np.asarray(res.results[i]["out"]).astype(np.float32).reshape(NPC, CO, H, W)
         for i in range(N_CORES)], axis=0)
    return out, res


def kernel(x, w, alpha_x, alpha_w):
    out, _ = run_on_hw(x, w, alpha_x, alpha_w)
    return out
